# revision 6
# baseline (speedup 1.0000x reference)
"""Trainium2 Bass kernel for nn_Model_17085379903564 (HiPPO-LegT multiscale
spectral forecaster).

Same linear-operator factorization as the previous version (see git/bak
docstring): per scale, W2 = U @ V (SVD, rank 128), P = V @ wfold,
g = f.T @ U, y = (g @ P - mu x tp) * std * ws, dec += y @ EmT, out =
dec + (bias*std + mean); per-core n-shard of V/w gives partial decs
that the host sums.

This version targets the DMA bottleneck found in the 45us baseline
trace (weights were shredded into 256-512B descriptors; PE idled until
t=18us):
  * every DRAM buffer is pre-swizzled host-side into partition-major
    [128, c, F] layout so each dma_start is 128 contiguous >=512B
    descriptors (line rate across the 16 SDMA engines);
  * spectral operands (V, wfold, U, EmT) travel as fp8e4 with power-2
    prescales (the dec path is ~0.12% of output norm, so fp8's ~2.4%
    relative error costs ~3e-5 end-to-end; measured 1.56e-3 total,
    dominated by bf16 input rounding, vs the 2e-2 gate);
  * fp8 matmuls use DoubleRow perf mode (contraction 256/instr);
  * instance-norm stats come from the already-loaded bf16 ftx via
    ones-vector PE reductions (drops the baseline's 512KB f32 x load).
Per-core HBM traffic: ~2.1MB loads + 128KB store.
"""

from contextlib import ExitStack

import ml_dtypes
import numpy as np

import concourse.bacc as bacc
import concourse.bass as bass
import concourse.mybir as mybir
import concourse.tile as tile
from concourse.bass_utils import run_bass_kernel_spmd
from concourse.masks import make_identity

# ---- problem constants (hardcoded; kernel.py must be self-contained) ----
B_SZ = 4
SEQ_LEN = 1024
PRED_LEN = 512
E_IN = 32
N_ORD = 256
MODES = 32
MULTISCALE = (1, 2)
BE = B_SZ * E_IN            # 128
N_CORES = 8
NSL = N_ORD // N_CORES      # 32  n-rows per core
NK = 2 * NSL * MODES        # 2048 stage-2 rows per core (re+im folded)
RANK = 128

F32 = mybir.dt.float32
BF16 = mybir.dt.bfloat16
F8 = mybir.dt.float8e4
BF16_NP = np.dtype(ml_dtypes.bfloat16)
F8_NP = np.dtype(ml_dtypes.float8_e4m3)   # TRN E4M3: max +-240

# power-of-2 prescales keeping every fp8 operand in the normal range
CW = 2.0 ** 18    # wfold
CV = 2.0 ** 10    # V
CU = 2.0 ** 9     # U / su
CE = 2.0 ** 6     # EmT
CF = 2.0 ** 3     # fp8 copy of ftx
CG = 2.0 ** 12    # g2 = g * std * ws * CG
CY = 2.0 ** -33   # yT -> fp8
CTOT_INV = 2.0 ** -25          # 1/(CW*CV*CU*CE*CF*CG*CY)
C_TP = -512.0                  # tp2 scale: -CF*CG/2^6
C_MU = 64.0                    # mu_std_row scale: 2^6
USE_DR = True
DR = mybir.MatmulPerfMode.DoubleRow


# ---------------------------------------------------------------- constants
def _transition_lmu(N):
    Q = np.arange(N, dtype=np.float64)
    R = (2 * Q + 1)[:, None]
    j, i = np.meshgrid(Q, Q)
    A = np.where(i < j, -1.0, (-1.0) ** (i - j + 1)) * R
    Bv = ((-1.0) ** Q[:, None] * R)[:, 0]
    return A, Bv


def _bilinear(A, Bv, dt):
    I = np.eye(A.shape[0])
    M = I - (dt / 2.0) * A
    Ad = np.linalg.solve(M, I + (dt / 2.0) * A)
    Bd = np.linalg.solve(M, dt * Bv)
    return Ad, Bd


def _legendre_vander(x, N):
    P = np.zeros((N, x.shape[0]))
    P[0] = 1.0
    if N > 1:
        P[1] = x
    for n in range(1, N - 1):
        P[n + 1] = ((2 * n + 1) * x * P[n] - n * P[n - 1]) / (n + 1)
    return P.T


def _scale_consts(ms):
    """Per-scale constants: U (L,r), V (r, 2*N*MODES), EmT (N, P)."""
    L = ms * PRED_LEN
    A, Bv = _transition_lmu(N_ORD)
    Ad, Bd = _bilinear(A, Bv, 1.0 / L)
    vals = np.arange(0.0, 1.0, 1.0 / L)
    Em = _legendre_vander(1.0 - 2.0 * vals, N_ORD)
    G = np.empty((L, N_ORD))
    g = Bd.copy()
    for m in range(L):
        G[m] = g
        g = Ad @ g
    k = np.arange(MODES)
    z = np.exp(-2j * np.pi * k / L)
    zm = z[None, :] ** np.arange(L)[:, None]
    Gpre = np.cumsum(zm[:, None, :] * G[:, :, None], axis=0)
    W = zm[:, None, :] * Gpre[::-1]
    e = (2.0 - (k == 0)) / L * np.exp(2j * np.pi * k * (PRED_LEN - 1) / L)
    W2 = W * e[None, None, :]
    M = np.concatenate(
        [W2.real.reshape(L, -1), (-W2.imag).reshape(L, -1)], axis=1)
    Uf, sv, Vt = np.linalg.svd(M, full_matrices=False)
    U = np.ascontiguousarray(Uf[:, :RANK])
    V = sv[:RANK, None] * Vt[:RANK]         # (r, 2*N*MODES)
    return U, V, Em[-PRED_LEN:].T


def _pmajor(a, nchunk):
    """(nchunk*128, F) row-major -> (128, nchunk, F) partition-major."""
    F = a.shape[1]
    return np.ascontiguousarray(
        a.reshape(nchunk, 128, F).transpose(1, 0, 2))


_CONSTS = None


def _get_consts():
    global _CONSTS
    if _CONSTS is None:
        _CONSTS = [_scale_consts(ms) for ms in MULTISCALE]
    return _CONSTS


# ---------------------------------------------------------------- bass prog
def _build_nc():
    nc = bacc.Bacc("TRN2", target_bir_lowering=False, debug=False,
                   num_devices=N_CORES)
    p = {}

    def par(name, shape, dt, out=False):
        p[name] = nc.declare_dram_parameter(name, shape, dt, isOutput=out)

    par("ftx", [128, 8, 128], BF16)
    for s in (0, 1):
        lch = (s + 1) * 4
        par(f"vt{s}a", [128, 8, 128], F8)
        par(f"vt{s}b", [128, 8, 128], F8)
        for j in range(4):
            par(f"wt{s}{j}", [128, 4, 256], F8)
        par(f"u{s}", [128, lch, 128], F8)
        par(f"su{s}", [128, 1], BF16)
        par(f"emt{s}", [128, 2, 512], F8)
    par("mlpw", [1, 2], F32)
    par("mlpb", [1, 1], F32)
    par("out_dec", [128, 512], BF16, out=True)

    with tile.TileContext(nc, num_cores=N_CORES) as tc:
        _emit(nc, tc, p)
    nc.finalize()
    return nc


def _mm(nc, out, lhsT, rhs, start, stop):
    if USE_DR:
        nc.tensor.matmul(out, lhsT=lhsT, rhs=rhs, start=start, stop=stop,
                         perf_mode=DR)
    else:
        for j in range(lhsT.shape[1]):
            nc.tensor.matmul(out, lhsT=lhsT[:, j, :], rhs=rhs[:, j, :],
                             start=(start and j == 0),
                             stop=(stop and j == lhsT.shape[1] - 1))


def _emit(nc, tc, p):
    AF = mybir.ActivationFunctionType
    with ExitStack() as ctx:
        const = ctx.enter_context(tc.tile_pool(name="const", bufs=1))
        work = ctx.enter_context(tc.tile_pool(name="work", bufs=1))
        ps_p = ctx.enter_context(
            tc.tile_pool(name="ps_p", bufs=2, space="PSUM"))
        ps_g = ctx.enter_context(
            tc.tile_pool(name="ps_g", bufs=1, space="PSUM"))
        ps_tr = ctx.enter_context(
            tc.tile_pool(name="ps_tr", bufs=2, space="PSUM"))
        ps_y = ctx.enter_context(
            tc.tile_pool(name="ps_y", bufs=2, space="PSUM"))
        ps_dec = ctx.enter_context(
            tc.tile_pool(name="ps_dec", bufs=1, space="PSUM"))

        # ---- DMA streams (issue everything up front; FIFO per ring) ----
        # sync ring: the vt/wt bulk, ordered by P-matmul consumption
        vt, wt = {}, {}
        for s in (0, 1):
            for t in ("a", "b"):
                vt[s, t] = const.tile([128, 8, 128], F8, tag=f"vt{s}{t}",
                                      name=f"vt{s}{t}")
            for j in range(4):
                wt[s, j] = const.tile([128, 4, 256], F8, tag=f"wt{s}{j}",
                                      name=f"wt{s}{j}")
        for s in (0, 1):
            nc.sync.dma_start(vt[s, "a"][:], p[f"vt{s}a"][:, :, :])
            nc.sync.dma_start(wt[s, 0][:], p[f"wt{s}0"][:, :, :])
            nc.sync.dma_start(vt[s, "b"][:], p[f"vt{s}b"][:, :, :])
            nc.sync.dma_start(wt[s, 2][:], p[f"wt{s}2"][:, :, :])
        # scalar ring: second half of each wt + small operands
        u_t, su_t, emt_t = {}, {}, {}
        for s in (0, 1):
            lch = (s + 1) * 4
            u_t[s] = const.tile([128, lch, 128], F8, tag=f"u{s}", name=f"u{s}")
            su_t[s] = const.tile([128, 1], BF16, tag=f"su{s}", name=f"su{s}")
            emt_t[s] = const.tile([128, 2, 512], F8, tag=f"emt{s}",
                                  name=f"emt{s}")
        for s in (0, 1):
            nc.scalar.dma_start(wt[s, 1][:], p[f"wt{s}1"][:, :, :])
            nc.scalar.dma_start(u_t[s][:], p[f"u{s}"][:, :, :])
            nc.scalar.dma_start(su_t[s][:], p[f"su{s}"][:, :])
            nc.scalar.dma_start(wt[s, 3][:], p[f"wt{s}3"][:, :, :])
            nc.scalar.dma_start(emt_t[s][:], p[f"emt{s}"][:, :, :])
        # gpsimd (SWDGE): x and the mlp scalars
        ftx = const.tile([128, 8, 128], BF16, tag="ftx")
        nc.gpsimd.dma_start(ftx[:], p["ftx"][:, :, :])
        mlpw_sb = const.tile([1, 2], F32, tag="mlpw")
        nc.gpsimd.dma_start(mlpw_sb[:], p["mlpw"][:, :])
        mlpb_sb = const.tile([1, 1], F32, tag="mlpb")
        nc.gpsimd.dma_start(mlpb_sb[:], p["mlpb"][:, :])

        # ---- small SBUF constants ----
        ident_b = const.tile([128, 128], BF16, tag="ident_b")
        make_identity(nc, ident_b[:])
        ones_b = const.tile([128, 1], BF16, tag="ones_b")
        nc.vector.memset(ones_b[:], 1.0)
        ones_f = const.tile([1, 128], F32, tag="ones_f")
        nc.vector.memset(ones_f[:], 1.0)

        # fp8 copy of x for the g matmuls (scalar engine, off to the side)
        ftx8 = work.tile([128, 8, 128], F8, tag="ftx8")
        nc.scalar.activation(ftx8[:], ftx[:], AF.Copy, scale=CF)
        sq = work.tile([128, 8, 128], BF16, tag="sq")
        nc.vector.tensor_mul(sq[:], ftx[:], ftx[:])

        # ---- stats: PE ones-reductions over the time (partition) axis ----
        sum_ps = ps_tr.tile([1, 128], F32, tag="tr", name="sum_ps")
        for c in range(8):
            nc.tensor.matmul(sum_ps[:], lhsT=ones_b[:], rhs=ftx[:, c, :],
                             start=(c == 0), stop=(c == 7))
        sq_ps = ps_tr.tile([1, 128], F32, tag="tr", name="sq_ps")
        for c in range(8):
            nc.tensor.matmul(sq_ps[:], lhsT=ones_b[:], rhs=sq[:, c, :],
                             start=(c == 0), stop=(c == 7))
        mean_row = work.tile([1, 128], F32, tag="mean_row")
        nc.scalar.mul(mean_row[:], sum_ps[:], 1.0 / SEQ_LEN)
        ex2_row = work.tile([1, 128], F32, tag="ex2_row")
        nc.scalar.mul(ex2_row[:], sq_ps[:], 1.0 / SEQ_LEN)
        m2_row = work.tile([1, 128], F32, tag="m2_row")
        nc.vector.tensor_mul(m2_row[:], mean_row[:], mean_row[:])
        var_row = work.tile([1, 128], F32, tag="var_row")
        nc.vector.tensor_sub(var_row[:], ex2_row[:], m2_row[:])
        eps_t = work.tile([1, 1], F32, tag="eps_t")
        nc.vector.memset(eps_t[:], 1e-5)
        std_row = work.tile([1, 128], F32, tag="std_row")
        nc.scalar.activation(std_row[:], var_row[:], AF.Sqrt, bias=eps_t[:])
        # mu_std_row (bf16, x2^6) feeds the rank-1 norm correction
        msf_row = work.tile([1, 128], F32, tag="msf_row")
        nc.vector.tensor_mul(msf_row[:], mean_row[:], std_row[:])
        mu_std_row = work.tile([1, 128], BF16, tag="mu_std_row")
        nc.scalar.mul(mu_std_row[:], msf_row[:], C_MU)

        # column ([BE,1]) versions of mean/std via tiny PE outer products
        mean_ps = ps_tr.tile([128, 1], F32, tag="tr", name="mean_ps")
        nc.tensor.matmul(mean_ps[:], lhsT=mean_row[:], rhs=ones_f[:, 0:1])
        std_ps = ps_tr.tile([128, 1], F32, tag="tr", name="std_ps")
        nc.tensor.matmul(std_ps[:], lhsT=std_row[:], rhs=ones_f[:, 0:1])
        mean_col = work.tile([128, 1], F32, tag="mean_col")
        nc.vector.tensor_copy(mean_col[:], mean_ps[:])
        std_col = work.tile([128, 1], F32, tag="std_col")
        nc.vector.tensor_copy(std_col[:], std_ps[:])
        # broadcast mlp scalars to all partitions
        ws_ps = ps_tr.tile([128, 2], F32, tag="tr", name="ws_ps")
        nc.tensor.matmul(ws_ps[:], lhsT=ones_f[:], rhs=mlpw_sb[:])
        ws_sb = work.tile([128, 2], F32, tag="ws_sb")
        nc.vector.tensor_copy(ws_sb[:], ws_ps[:])
        bs_ps = ps_tr.tile([128, 1], F32, tag="tr", name="bs_ps")
        nc.tensor.matmul(bs_ps[:], lhsT=ones_f[:], rhs=mlpb_sb[:])
        bmu8 = work.tile([128, 1], F32, tag="bmu8")
        nc.vector.tensor_mul(bmu8[:], bs_ps[:], std_col[:])
        nc.vector.tensor_add(bmu8[:], bmu8[:], mean_col[:])
        nc.scalar.mul(bmu8[:], bmu8[:], 1.0 / N_CORES)
        # per-scale scales
        stdcg = work.tile([128, 1], F32, tag="stdcg")
        nc.scalar.mul(stdcg[:], std_col[:], CG)
        stdws = work.tile([128, 2], F32, tag="stdws")
        for s in (0, 1):
            nc.vector.tensor_mul(stdws[:, s:s + 1], stdcg[:],
                                 ws_sb[:, s:s + 1])
        mlpw_neg = work.tile([1, 2], F32, tag="mlpw_neg")
        nc.scalar.mul(mlpw_neg[:], mlpw_sb[:], C_TP)

        dec_ps = ps_dec.tile([BE, PRED_LEN], F32, tag="dec")

        for s in (0, 1):
            lch = (s + 1) * 4
            # P = V' @ w'  (8 DoubleRow matmuls, nk = 2048)
            p_ps = ps_p.tile([128, 256], F32, tag="p")
            for i in range(8):
                vtile = vt[s, "a" if i < 4 else "b"]
                wtile = wt[s, i // 2]
                _mm(nc, p_ps[:],
                    vtile[:, 2 * (i % 4):2 * (i % 4) + 2, :],
                    wtile[:, 2 * (i % 2):2 * (i % 2) + 2, :],
                    start=(i == 0), stop=(i == 7))
            p_sb = work.tile([128, 256], BF16, tag="p_sb", name=f"p{s}")
            nc.vector.tensor_copy(p_sb[:], p_ps[:])
            # tp = su' @ P  (negated+ws-scaled into tp2)
            tp_ps = ps_tr.tile([1, 256], F32, tag="tr", name=f"tp{s}")
            nc.tensor.matmul(tp_ps[:], lhsT=su_t[s][:], rhs=p_sb[:])
            tp2 = work.tile([1, 256], BF16, tag="tp2", name=f"tp2{s}")
            nc.scalar.activation(tp2[:], tp_ps[:], AF.Copy,
                                 scale=mlpw_neg[:, s:s + 1])
            # g = f8.T @ u'   (time chunks: last lch of ftx8)
            g_ps = ps_g.tile([128, 128], F32, tag="g")
            j0 = 8 - lch
            for d in range(lch // 2):
                _mm(nc, g_ps[:],
                    ftx8[:, j0 + 2 * d:j0 + 2 * d + 2, :],
                    u_t[s][:, 2 * d:2 * d + 2, :],
                    start=(d == 0), stop=(d == lch // 2 - 1))
            g2 = work.tile([128, 128], BF16, tag="g2", name=f"g2{s}")
            nc.scalar.activation(g2[:], g_ps[:], AF.Copy,
                                 scale=stdws[:, s:s + 1])
            tr_ps = ps_tr.tile([128, 128], BF16, tag="tr")
            nc.tensor.transpose(tr_ps[:], g2[:], ident_b[:])
            g2T = work.tile([128, 128], BF16, tag="g2T", name=f"g2T{s}")
            nc.vector.tensor_copy(g2T[:], tr_ps[:])
            # yT[o, be] = P^T @ g2T - (ws) tp x mu_std ; -> fp8
            yt8 = work.tile([128, 2, 128], F8, tag="yt8", name=f"yt8{s}")
            for och in (0, 1):
                y_ps = ps_y.tile([128, 128], F32, tag="y")
                nc.tensor.matmul(y_ps[:],
                                 lhsT=p_sb[:, och * 128:och * 128 + 128],
                                 rhs=g2T[:], start=True, stop=False)
                nc.tensor.matmul(y_ps[:],
                                 lhsT=tp2[:, och * 128:och * 128 + 128],
                                 rhs=mu_std_row[:], start=False, stop=True)
                nc.scalar.activation(yt8[:, och, :], y_ps[:], AF.Copy,
                                     scale=CY)
            # dec += yT'' @ EmT'  (one DoubleRow matmul, o = 256)
            _mm(nc, dec_ps[:], yt8[:], emt_t[s][:],
                start=(s == 0), stop=(s == 1))

        out_sb = work.tile([BE, PRED_LEN], BF16, tag="out")
        nc.scalar.activation(out_sb[:], dec_ps[:], AF.Identity,
                             bias=bmu8[:], scale=CTOT_INV)
        nc.sync.dma_start(p["out_dec"][:, :], out_sb[:])


_NC = None


def _get_nc():
    global _NC
    if _NC is None:
        _NC = _build_nc()
    return _NC


# ---------------------------------------------------------------- host side
_CONST_MAPS = None


def _const_maps():
    global _CONST_MAPS
    if _CONST_MAPS is None:
        consts = _get_consts()
        _CONST_MAPS = []
        for c in range(N_CORES):
            m = {}
            for s in (0, 1):
                U, V, EmT = consts[s]
                lch = (s + 1) * 4
                vs = V.reshape(RANK, 2, N_ORD, MODES)[
                    :, :, c * NSL:(c + 1) * NSL, :]          # (r,2,32,32)
                vtm = _pmajor(
                    (vs.reshape(RANK, NK).T * CV).astype(F8_NP), 16)
                m[f"vt{s}a"] = np.ascontiguousarray(vtm[:, :8])
                m[f"vt{s}b"] = np.ascontiguousarray(vtm[:, 8:])
                m[f"u{s}"] = _pmajor((U * CU).astype(F8_NP), lch)
                m[f"su{s}"] = np.ascontiguousarray(
                    (U.sum(axis=0) * CU).reshape(128, 1)).astype(BF16_NP)
                m[f"emt{s}"] = _pmajor((EmT * CE).astype(F8_NP), 2)
            _CONST_MAPS.append(m)
    return _CONST_MAPS


def _in_maps(x_enc, spec_w_real, spec_w_imag, mlp_weight, mlp_bias):
    ftx = _pmajor(
        x_enc.transpose(1, 0, 2).reshape(SEQ_LEN, BE).astype(BF16_NP), 8)
    mw = np.asarray(mlp_weight, np.float32).reshape(1, 2)
    mb = np.asarray(mlp_bias, np.float32).reshape(1, 1)
    shared = {"ftx": ftx, "mlpw": mw, "mlpb": mb}

    maps = []
    for c in range(N_CORES):
        n0 = c * NSL
        m = dict(shared)
        m.update(_const_maps()[c])
        for s in (0, 1):
            wr = spec_w_real[s, n0:n0 + NSL].transpose(0, 2, 1).reshape(
                NK // 2, N_ORD)
            wi = spec_w_imag[s, n0:n0 + NSL].transpose(0, 2, 1).reshape(
                NK // 2, N_ORD)
            wf = _pmajor(
                (np.concatenate([wr, wi], axis=0) * CW).astype(F8_NP), 16)
            for j in range(4):
                m[f"wt{s}{j}"] = np.ascontiguousarray(
                    wf[:, 4 * j:4 * j + 4])
        maps.append(m)
    return maps


def kernel(x_enc, spec_w_real, spec_w_imag, mlp_weight, mlp_bias,
           _trace=False, _trace_kwargs=None):
    x_enc = np.asarray(x_enc, np.float32)
    spec_w_real = np.asarray(spec_w_real, np.float32)
    spec_w_imag = np.asarray(spec_w_imag, np.float32)
    maps = _in_maps(x_enc, spec_w_real, spec_w_imag, mlp_weight, mlp_bias)
    nc = _get_nc()
    res = run_bass_kernel_spmd(nc, maps, list(range(N_CORES)),
                               trace=_trace, **(_trace_kwargs or {}))
    # out_dec[c] = partial dec over core c's n-shard; unshard = sum
    full = np.sum([res.results[c]["out_dec"].astype(np.float32)
                   for c in range(N_CORES)], axis=0)
    out = np.ascontiguousarray(
        full.reshape(B_SZ, E_IN, PRED_LEN).transpose(0, 2, 1), np.float32)
    if _trace:
        return out, res
    return out


# revision 7
# speedup vs baseline: 1.0333x; 1.0333x over previous
"""Trainium2 Bass kernel for nn_Model_17085379903564 (HiPPO-LegT multiscale
spectral forecaster).

Same linear-operator factorization as the previous version (see git/bak
docstring): per scale, W2 = U @ V (SVD, rank 128), P = V @ wfold,
g = f.T @ U, y = (g @ P - mu x tp) * std * ws, dec += y @ EmT, out =
dec + (bias*std + mean); per-core n-shard of V/w gives partial decs
that the host sums.

This version targets the DMA bottleneck found in the 45us baseline
trace (weights were shredded into 256-512B descriptors; PE idled until
t=18us):
  * every DRAM buffer is pre-swizzled host-side into partition-major
    [128, c, F] layout so each dma_start is 128 contiguous >=512B
    descriptors (line rate across the 16 SDMA engines);
  * spectral operands (V, wfold, U, EmT) travel as fp8e4 with power-2
    prescales (the dec path is ~0.12% of output norm, so fp8's ~2.4%
    relative error costs ~3e-5 end-to-end; measured 1.56e-3 total,
    dominated by bf16 input rounding, vs the 2e-2 gate);
  * fp8 matmuls use DoubleRow perf mode (contraction 256/instr);
  * instance-norm stats come from the already-loaded bf16 ftx via
    ones-vector PE reductions (drops the baseline's 512KB f32 x load).
Per-core HBM traffic: ~2.1MB loads + 128KB store.
"""

from contextlib import ExitStack

import ml_dtypes
import numpy as np

import concourse.bacc as bacc
import concourse.bass as bass
import concourse.mybir as mybir
import concourse.tile as tile
from concourse.bass_utils import run_bass_kernel_spmd
from concourse.masks import make_identity

# ---- problem constants (hardcoded; kernel.py must be self-contained) ----
B_SZ = 4
SEQ_LEN = 1024
PRED_LEN = 512
E_IN = 32
N_ORD = 256
MODES = 32
MULTISCALE = (1, 2)
BE = B_SZ * E_IN            # 128
N_CORES = 8
NSL = N_ORD // N_CORES      # 32  n-rows per core
NK = 2 * NSL * MODES        # 2048 stage-2 rows per core (re+im folded)
RANK = 128

F32 = mybir.dt.float32
BF16 = mybir.dt.bfloat16
F8 = mybir.dt.float8e4
BF16_NP = np.dtype(ml_dtypes.bfloat16)
F8_NP = np.dtype(ml_dtypes.float8_e4m3)   # TRN E4M3: max +-240

# power-of-2 prescales keeping every fp8 operand in the normal range
CW = 2.0 ** 18    # wfold
CV = 2.0 ** 10    # V
CU = 2.0 ** 9     # U / su
CE = 2.0 ** 6     # EmT
CF = 2.0 ** 3     # fp8 copy of ftx
CG = 2.0 ** 12    # g2 = g * std * ws * CG
CY = 2.0 ** -33   # yT -> fp8
CTOT_INV = 2.0 ** -25          # 1/(CW*CV*CU*CE*CF*CG*CY)
C_TP = -0.125                  # tp2 scale: -CF/2^6
C_MU = 64.0                    # mu_std_row scale: 2^6
USE_DR = True
DR = mybir.MatmulPerfMode.DoubleRow


# ---------------------------------------------------------------- constants
def _transition_lmu(N):
    Q = np.arange(N, dtype=np.float64)
    R = (2 * Q + 1)[:, None]
    j, i = np.meshgrid(Q, Q)
    A = np.where(i < j, -1.0, (-1.0) ** (i - j + 1)) * R
    Bv = ((-1.0) ** Q[:, None] * R)[:, 0]
    return A, Bv


def _bilinear(A, Bv, dt):
    I = np.eye(A.shape[0])
    M = I - (dt / 2.0) * A
    Ad = np.linalg.solve(M, I + (dt / 2.0) * A)
    Bd = np.linalg.solve(M, dt * Bv)
    return Ad, Bd


def _legendre_vander(x, N):
    P = np.zeros((N, x.shape[0]))
    P[0] = 1.0
    if N > 1:
        P[1] = x
    for n in range(1, N - 1):
        P[n + 1] = ((2 * n + 1) * x * P[n] - n * P[n - 1]) / (n + 1)
    return P.T


def _scale_consts(ms):
    """Per-scale constants: U (L,r), V (r, 2*N*MODES), EmT (N, P)."""
    L = ms * PRED_LEN
    A, Bv = _transition_lmu(N_ORD)
    Ad, Bd = _bilinear(A, Bv, 1.0 / L)
    vals = np.arange(0.0, 1.0, 1.0 / L)
    Em = _legendre_vander(1.0 - 2.0 * vals, N_ORD)
    G = np.empty((L, N_ORD))
    g = Bd.copy()
    for m in range(L):
        G[m] = g
        g = Ad @ g
    k = np.arange(MODES)
    z = np.exp(-2j * np.pi * k / L)
    zm = z[None, :] ** np.arange(L)[:, None]
    Gpre = np.cumsum(zm[:, None, :] * G[:, :, None], axis=0)
    W = zm[:, None, :] * Gpre[::-1]
    e = (2.0 - (k == 0)) / L * np.exp(2j * np.pi * k * (PRED_LEN - 1) / L)
    W2 = W * e[None, None, :]
    M = np.concatenate(
        [W2.real.reshape(L, -1), (-W2.imag).reshape(L, -1)], axis=1)
    Uf, sv, Vt = np.linalg.svd(M, full_matrices=False)
    U = np.ascontiguousarray(Uf[:, :RANK])
    V = sv[:RANK, None] * Vt[:RANK]         # (r, 2*N*MODES)
    return U, V, Em[-PRED_LEN:].T


def _pmajor(a, nchunk):
    """(nchunk*128, F) row-major -> (128, nchunk, F) partition-major."""
    F = a.shape[1]
    return np.ascontiguousarray(
        a.reshape(nchunk, 128, F).transpose(1, 0, 2))


_CONSTS = None


def _get_consts():
    global _CONSTS
    if _CONSTS is None:
        _CONSTS = [_scale_consts(ms) for ms in MULTISCALE]
    return _CONSTS


# ---------------------------------------------------------------- bass prog
def _build_nc():
    nc = bacc.Bacc("TRN2", target_bir_lowering=False, debug=False,
                   num_devices=N_CORES)
    p = {}

    def par(name, shape, dt, out=False):
        p[name] = nc.declare_dram_parameter(name, shape, dt, isOutput=out)

    par("auxb", [128, 1026], BF16)
    par("aux8", [128, 3584], F8)
    par("bg0", [128, 16, 384], F8)
    par("bg1", [128, 16, 384], F8)
    par("mlps", [1, 3], F32)
    par("out_dec", [128, 512], BF16, out=True)

    with tile.TileContext(nc, num_cores=N_CORES) as tc:
        _emit(nc, tc, p)
    nc.finalize()
    return nc


def _mm(nc, out, lhsT, rhs, start, stop):
    if USE_DR:
        nc.tensor.matmul(out, lhsT=lhsT, rhs=rhs, start=start, stop=stop,
                         perf_mode=DR)
    else:
        for j in range(lhsT.shape[1]):
            nc.tensor.matmul(out, lhsT=lhsT[:, j, :], rhs=rhs[:, j, :],
                             start=(start and j == 0),
                             stop=(stop and j == lhsT.shape[1] - 1))


def _emit(nc, tc, p):
    AF = mybir.ActivationFunctionType
    OP = mybir.AluOpType
    with ExitStack() as ctx:
        const = ctx.enter_context(tc.tile_pool(name="const", bufs=1))
        work = ctx.enter_context(tc.tile_pool(name="work", bufs=1))
        ps_p = ctx.enter_context(
            tc.tile_pool(name="ps_p", bufs=2, space="PSUM"))
        ps_g = ctx.enter_context(
            tc.tile_pool(name="ps_g", bufs=1, space="PSUM"))
        ps_tr = ctx.enter_context(
            tc.tile_pool(name="ps_tr", bufs=2, space="PSUM"))
        ps_y = ctx.enter_context(
            tc.tile_pool(name="ps_y", bufs=2, space="PSUM"))
        ps_dec = ctx.enter_context(
            tc.tile_pool(name="ps_dec", bufs=1, space="PSUM"))

        # ---- 5 consolidated loads (2 rings, consumption order) ----
        auxb = const.tile([128, 1026], BF16, tag="auxb")
        nc.scalar.dma_start(auxb[:], p["auxb"][:, :])
        aux8 = const.tile([128, 3584], F8, tag="aux8")
        nc.scalar.dma_start(aux8[:], p["aux8"][:, :])
        mlps = const.tile([1, 3], F32, tag="mlps")
        nc.scalar.dma_start(mlps[:], p["mlps"][:, :])
        bg = {}
        for s in (0, 1):
            bg[s] = const.tile([128, 16, 384], F8, tag=f"bg{s}",
                               name=f"bg{s}")
            nc.sync.dma_start(bg[s][:], p[f"bg{s}"][:, :, :])

        ftxv = auxb[:, 0:1024]
        u_view = {0: aux8[:, 0:512].rearrange("p (c f) -> p c f", f=128),
                  1: aux8[:, 512:1536].rearrange("p (c f) -> p c f", f=128)}
        emt_view = {
            0: aux8[:, 1536:2560].rearrange("p (c f) -> p c f", f=512),
            1: aux8[:, 2560:3584].rearrange("p (c f) -> p c f", f=512)}

        ident_b = const.tile([128, 128], BF16, tag="ident_b")
        make_identity(nc, ident_b[:])
        ones_b = const.tile([128, 1], BF16, tag="ones_b")
        nc.vector.memset(ones_b[:], 1.0)
        ones_f = const.tile([1, 128], F32, tag="ones_f")
        nc.vector.memset(ones_f[:], 1.0)
        eps_t = work.tile([1, 1], F32, tag="eps_t")
        nc.vector.memset(eps_t[:], 1e-5)

        # fp8 copy of x for the g matmuls
        ftx8 = work.tile([128, 8, 128], F8, tag="ftx8")
        nc.vector.tensor_scalar_mul(ftx8[:], ftxv, CF)
        sq = work.tile([128, 1024], BF16, tag="sq")
        nc.vector.tensor_mul(sq[:], ftxv, ftxv)

        # ---- stats: 4 wide PE ones-reductions + small vector folds ----
        sum_ps = ps_tr.tile([1, 512], F32, tag="tr", name="sum_ps")
        nc.tensor.matmul(sum_ps[:], lhsT=ones_b[:], rhs=auxb[:, 0:512],
                         start=True, stop=False)
        nc.tensor.matmul(sum_ps[:], lhsT=ones_b[:], rhs=auxb[:, 512:1024],
                         start=False, stop=True)
        sum4 = work.tile([1, 512], F32, tag="sum4")
        nc.vector.tensor_copy(sum4[:], sum_ps[:])
        sq_ps = ps_tr.tile([1, 512], F32, tag="tr", name="sq_ps")
        nc.tensor.matmul(sq_ps[:], lhsT=ones_b[:], rhs=sq[:, 0:512],
                         start=True, stop=False)
        nc.tensor.matmul(sq_ps[:], lhsT=ones_b[:], rhs=sq[:, 512:1024],
                         start=False, stop=True)
        sq4 = work.tile([1, 512], F32, tag="sq4")
        nc.vector.tensor_copy(sq4[:], sq_ps[:])
        for t in (sum4, sq4):
            nc.vector.tensor_add(t[:, 0:128], t[:, 0:128], t[:, 128:256])
            nc.vector.tensor_add(t[:, 256:384], t[:, 256:384],
                                 t[:, 384:512])
            nc.vector.tensor_add(t[:, 0:128], t[:, 0:128], t[:, 256:384])
        mean_row = work.tile([1, 128], F32, tag="mean_row")
        nc.vector.tensor_scalar_mul(mean_row[:], sum4[:, 0:128],
                                    1.0 / SEQ_LEN)
        ex2_row = work.tile([1, 128], F32, tag="ex2_row")
        nc.vector.tensor_scalar_mul(ex2_row[:], sq4[:, 0:128],
                                    1.0 / SEQ_LEN)
        m2_row = work.tile([1, 128], F32, tag="m2_row")
        nc.vector.tensor_mul(m2_row[:], mean_row[:], mean_row[:])
        var_row = work.tile([1, 128], F32, tag="var_row")
        nc.vector.tensor_sub(var_row[:], ex2_row[:], m2_row[:])
        std_row = work.tile([1, 128], F32, tag="std_row")
        nc.scalar.activation(std_row[:], var_row[:], AF.Sqrt,
                             bias=eps_t[:])
        msf_row = work.tile([1, 128], F32, tag="msf_row")
        nc.vector.tensor_mul(msf_row[:], mean_row[:], std_row[:])
        mu_std_row = work.tile([1, 128], BF16, tag="mu_std_row")
        nc.vector.tensor_scalar_mul(mu_std_row[:], msf_row[:], C_MU)

        # column ([BE,1]) mean/std + mlp broadcasts via tiny PE matmuls
        mean_ps = ps_tr.tile([128, 1], F32, tag="tr", name="mean_ps")
        nc.tensor.matmul(mean_ps[:], lhsT=mean_row[:], rhs=ones_f[:, 0:1])
        std_ps = ps_tr.tile([128, 1], F32, tag="tr", name="std_ps")
        nc.tensor.matmul(std_ps[:], lhsT=std_row[:], rhs=ones_f[:, 0:1])
        mean_col = work.tile([128, 1], F32, tag="mean_col")
        nc.vector.tensor_copy(mean_col[:], mean_ps[:])
        std_col = work.tile([128, 1], F32, tag="std_col")
        nc.vector.tensor_copy(std_col[:], std_ps[:])
        ws_ps = ps_tr.tile([128, 2], F32, tag="tr", name="ws_ps")
        nc.tensor.matmul(ws_ps[:], lhsT=ones_f[:], rhs=mlps[:, 0:2])
        ws_sb = work.tile([128, 2], F32, tag="ws_sb")
        nc.vector.tensor_copy(ws_sb[:], ws_ps[:])
        bs_ps = ps_tr.tile([128, 1], F32, tag="tr", name="bs_ps")
        nc.tensor.matmul(bs_ps[:], lhsT=ones_f[:], rhs=mlps[:, 2:3])
        bmu8 = work.tile([128, 1], F32, tag="bmu8")
        nc.vector.tensor_mul(bmu8[:], bs_ps[:], std_col[:])
        nc.vector.tensor_add(bmu8[:], bmu8[:], mean_col[:])
        nc.vector.tensor_scalar_mul(bmu8[:], bmu8[:], 1.0 / N_CORES)
        stdws = work.tile([128, 2], F32, tag="stdws")
        for s in (0, 1):
            nc.vector.tensor_scalar_mul(stdws[:, s:s + 1], std_col[:],
                                        ws_sb[:, s:s + 1])
        mlpw_neg = work.tile([1, 2], F32, tag="mlpw_neg")
        nc.vector.tensor_scalar_mul(mlpw_neg[:], mlps[:, 0:2], C_TP)

        dec_ps = ps_dec.tile([BE, PRED_LEN], F32, tag="dec")

        for s in (0, 1):
            lch = (s + 1) * 4
            # P = V' @ w'  (8 DoubleRow matmuls over interleaved bg)
            p_ps = ps_p.tile([128, 256], F32, tag="p")
            for i in range(8):
                _mm(nc, p_ps[:], bg[s][:, 2 * i:2 * i + 2, 0:128],
                    bg[s][:, 2 * i:2 * i + 2, 128:384],
                    start=(i == 0), stop=(i == 7))
            p_sb = work.tile([128, 256], BF16, tag="p_sb", name=f"p{s}")
            nc.vector.tensor_copy(p_sb[:], p_ps[:])
            # tp = su' @ P ; tp2 = -ws*CF/2^6 * tp
            tp_ps = ps_tr.tile([1, 256], F32, tag="tr", name=f"tp{s}")
            nc.tensor.matmul(tp_ps[:], lhsT=auxb[:, 1024 + s:1025 + s],
                             rhs=p_sb[:])
            tp2 = work.tile([1, 256], BF16, tag="tp2", name=f"tp2{s}")
            nc.vector.tensor_scalar_mul(tp2[:], tp_ps[:],
                                        mlpw_neg[:, s:s + 1])
            # g = f8.T @ u'
            g_ps = ps_g.tile([128, 128], F32, tag="g")
            j0 = (8 - lch) // 2
            for d in range(lch // 2):
                _mm(nc, g_ps[:],
                    ftx8[:, 2 * (j0 + d):2 * (j0 + d) + 2, :],
                    u_view[s][:, 2 * d:2 * d + 2, :],
                    start=(d == 0), stop=(d == lch // 2 - 1))
            g2 = work.tile([128, 128], BF16, tag="g2", name=f"g2{s}")
            nc.vector.tensor_scalar_mul(g2[:], g_ps[:], stdws[:, s:s + 1])
            tr_ps = ps_tr.tile([128, 128], BF16, tag="tr")
            nc.tensor.transpose(tr_ps[:], g2[:], ident_b[:])
            g2T = work.tile([128, 128], BF16, tag="g2T", name=f"g2T{s}")
            nc.vector.tensor_copy(g2T[:], tr_ps[:])
            # yT[o, be] = P^T @ g2T - ws tp x mu_std ; -> fp8 (*CY*CG)
            yt8 = work.tile([128, 2, 128], F8, tag="yt8", name=f"yt8{s}")
            for och in (0, 1):
                y_ps = ps_y.tile([128, 128], F32, tag="y")
                nc.tensor.matmul(y_ps[:],
                                 lhsT=p_sb[:, och * 128:och * 128 + 128],
                                 rhs=g2T[:], start=True, stop=False)
                nc.tensor.matmul(y_ps[:],
                                 lhsT=tp2[:, och * 128:och * 128 + 128],
                                 rhs=mu_std_row[:], start=False, stop=True)
                nc.vector.tensor_scalar_mul(yt8[:, och, :], y_ps[:],
                                            CY * CG)
            # dec += yT'' @ EmT'  (one DoubleRow matmul, o = 256)
            _mm(nc, dec_ps[:], yt8[:], emt_view[s],
                start=(s == 0), stop=(s == 1))

        out_sb = work.tile([BE, PRED_LEN], BF16, tag="out")
        nc.vector.tensor_scalar(out_sb[:], dec_ps[:], CTOT_INV, bmu8[:],
                                op0=OP.mult, op1=OP.add)
        nc.sync.dma_start(p["out_dec"][:, :], out_sb[:])


_NC = None


def _get_nc():
    global _NC
    if _NC is None:
        _NC = _build_nc()
    return _NC


# ---------------------------------------------------------------- host side
_CONST_MAPS = None


def _const_maps():
    global _CONST_MAPS
    if _CONST_MAPS is None:
        consts = _get_consts()
        _CONST_MAPS = []
        for c in range(N_CORES):
            m = {}
            aux8 = np.empty((128, 3584), F8_NP)
            sub = np.empty((128, 2), BF16_NP)
            vt_pm = {}
            for s in (0, 1):
                U, V, EmT = consts[s]
                lch = (s + 1) * 4
                vs = V.reshape(RANK, 2, N_ORD, MODES)[
                    :, :, c * NSL:(c + 1) * NSL, :]
                vt_pm[s] = _pmajor(
                    (vs.reshape(RANK, NK).T * CV).astype(F8_NP), 16)
                u_pm = _pmajor((U * CU).astype(F8_NP), lch)
                o = 0 if s == 0 else 512
                aux8[:, o:o + lch * 128] = u_pm.reshape(128, lch * 128)
                aux8[:, 1536 + 1024 * s:2560 + 1024 * s] = _pmajor(
                    (EmT * CE).astype(F8_NP), 2).reshape(128, 1024)
                sub[:, s] = (U.sum(axis=0) * CU).astype(BF16_NP)
            m["aux8"] = aux8
            m["_sub"] = sub
            m["_vt"] = vt_pm
            _CONST_MAPS.append(m)
    return _CONST_MAPS


def _in_maps(x_enc, spec_w_real, spec_w_imag, mlp_weight, mlp_bias):
    ftx = _pmajor(
        x_enc.transpose(1, 0, 2).reshape(SEQ_LEN, BE).astype(BF16_NP), 8)
    mlps = np.concatenate([
        np.asarray(mlp_weight, np.float32).reshape(1, 2),
        np.asarray(mlp_bias, np.float32).reshape(1, 1)], axis=1)

    maps = []
    for c in range(N_CORES):
        cm = _const_maps()[c]
        n0 = c * NSL
        auxb = np.empty((128, 1026), BF16_NP)
        auxb[:, 0:1024] = ftx.reshape(128, 1024)
        auxb[:, 1024:1026] = cm["_sub"]
        m = {"auxb": auxb, "aux8": cm["aux8"], "mlps": mlps}
        for s in (0, 1):
            wr = spec_w_real[s, n0:n0 + NSL].transpose(0, 2, 1).reshape(
                NK // 2, N_ORD)
            wi = spec_w_imag[s, n0:n0 + NSL].transpose(0, 2, 1).reshape(
                NK // 2, N_ORD)
            wf = _pmajor(
                (np.concatenate([wr, wi], axis=0) * CW).astype(F8_NP), 16)
            bgs = np.empty((128, 16, 384), F8_NP)
            bgs[:, :, 0:128] = cm["_vt"][s]
            bgs[:, :, 128:384] = wf
            m[f"bg{s}"] = bgs
        maps.append(m)
    return maps


def kernel(x_enc, spec_w_real, spec_w_imag, mlp_weight, mlp_bias,
           _trace=False, _trace_kwargs=None):
    x_enc = np.asarray(x_enc, np.float32)
    spec_w_real = np.asarray(spec_w_real, np.float32)
    spec_w_imag = np.asarray(spec_w_imag, np.float32)
    maps = _in_maps(x_enc, spec_w_real, spec_w_imag, mlp_weight, mlp_bias)
    nc = _get_nc()
    res = run_bass_kernel_spmd(nc, maps, list(range(N_CORES)),
                               trace=_trace, **(_trace_kwargs or {}))
    # out_dec[c] = partial dec over core c's n-shard; unshard = sum
    full = np.sum([res.results[c]["out_dec"].astype(np.float32)
                   for c in range(N_CORES)], axis=0)
    out = np.ascontiguousarray(
        full.reshape(B_SZ, E_IN, PRED_LEN).transpose(0, 2, 1), np.float32)
    if _trace:
        return out, res
    return out
